# revision 51
# baseline (speedup 1.0000x reference)
"""Trainium2 Bass kernel for causal multi-head attention.

Problem: B=2, T=4096, D=768, H=12 heads, d_k=64, causal mask.
Sharding: 8 cores = 2 batches x 4 head-groups (3 heads each).

v2 design:
- All inputs shipped fp16 (x pre-transposed on host into [6, 128, T'] chunks).
- x is uploaded T-sharded (each core gets a distinct 1024-token slice of its
  batch) and AllGather'd on-device across the 4-core batch group, cutting
  host->device x bytes 4x vs replicating.
- All matmuls run fp16 operands (1 cyc/row + fast weight load), fp32 PSUM.
- Attention uses transposed scores (S^T = k q^T) so softmax statistics land
  matmul-friendly with no P-transposes; denominators via an appended
  ones-column in v (row 64 of the PV accumulation).
- Softmax normalization via reciprocal_approx_fast on a broadcast tile
  (the old single-lane PSUM reciprocal was 3.3us per call).
- Projections for chunk i+1 and the deferred output projection of chunk i-1
  are interleaved into attention's matmul stream as filler, keeping TensorE
  dense so the HAM clock gate stays at full rate.
- Partial outputs are ReduceScatter'd on-device (fp16) per 512-token chunk,
  overlapped with compute; each core returns a disjoint [8, 128, 768] slab,
  cutting device->host bytes 16x vs fp32 replicated partials.
- Host folds the v-bias through W_out and adds b_out. k-bias is dropped
  (softmax is invariant to per-query score shifts).

Self-contained: hardcodes all shapes; only imports the concourse runtime.
"""

import sys

sys.path.insert(0, "/opt/trn_rl_repo")

from contextlib import ExitStack

import numpy as np

import concourse.bass as bass
import concourse.mybir as mybir
import concourse.tile as tile
from concourse import bacc
from concourse.bass_utils import run_bass_kernel_spmd

F16 = mybir.dt.float16
F32 = mybir.dt.float32

B, T, D = 2, 4096, 768
H, DK = 12, 64
HPC = 3          # heads per core
N_CORES = 8
ICH_W = 512      # i-chunk width (queries per outer step)
JB_W = 128       # j-block width (keys per matmul)
KT = D // 128    # contraction tiles for projections
VW = HPC * DK    # v projection width
GROUPS = [[0, 1, 2, 3], [4, 5, 6, 7]]
# ReduceScatter windows over token rows: few floors, small tail window
RS_ROWS = [(0, 1024), (1024, 2048), (2048, 3072), (3072, 3584),
           (3584, 4096)]

USE_COLL = True  # AllGather x + ReduceScatter out on-device


def build_program(t=T, use_coll=USE_COLL):
    """Build the SPMD Bass program for one core (all cores identical)."""
    n_ich = t // ICH_W
    n_tch = t // 128
    tpc = t // 4                  # tokens per core in the x shard (AG mode)

    nc = bacc.Bacc("TRN2", target_bir_lowering=False, debug=False,
                   num_devices=N_CORES)

    sw = t // 16                  # strip width (4 strips per core shard)
    if use_coll:
        # core c's local strip s = global strip 4s+c (so AG#s delivers the
        # 4 consecutive global strips 4s..4s+3 = chunks 2s, 2s+1)
        x_d = nc.dram_tensor("x", [4, KT, 128, sw], F16,
                             kind="ExternalInput").ap()
    else:
        x_d = nc.dram_tensor("x", [KT, 128, t], F16,
                             kind="ExternalInput").ap()
    # qk projection weights, 4 chunks of 128 output channels:
    # ch0=[q0|q1] ch1=[k0|k1] ch2=[q2|k2] ch3=[k2|q2]
    wqk_d = nc.dram_tensor("wqk", [D, 512], F16, kind="ExternalInput").ap()
    bqk_d = nc.dram_tensor("bqk", [512], F32, kind="ExternalInput").ap()
    wv_d = nc.dram_tensor("wv", [D, VW], F16, kind="ExternalInput").ap()
    wout_d = nc.dram_tensor("wout", [VW, D], F16, kind="ExternalInput").ap()
    if use_coll:
        out_d = nc.dram_tensor("out", [t // 4, D], F16,
                               kind="ExternalOutput").ap()
    else:
        out_d = nc.dram_tensor("out", [t, D], F16, kind="ExternalOutput").ap()
    # ReduceScatter row windows keyed by the closing row
    row_close = {r1: (r0, r1) for r0, r1 in RS_ROWS}

    with tile.TileContext(nc) as tc, ExitStack() as top:
        consts = top.enter_context(tc.tile_pool(name="consts", bufs=1))
        persist = top.enter_context(tc.tile_pool(name="persist", bufs=1))

        if use_coll:
            # kick the AllGather pipeline off first: bounce the input shard
            # strip-by-strip and trigger the 4 strip AGs ASAP
            dram = top.enter_context(
                tc.tile_pool(name="dram", bufs=1, space="DRAM"))
            xb = dram.tile([4, KT, 128, sw], F16)
            xg = dram.tile([4, 4, KT, 128, sw], F16)
            ob = dram.tile([t, D], F16)
            rsq = dram.tile([t // 4, D], F16)
            for s in range(4):
                nc.gpsimd.dma_start(xb[s], x_d[s])
                nc.gpsimd.collective_compute(
                    "AllGather", mybir.AluOpType.bypass,
                    replica_groups=GROUPS,
                    ins=[xb[s].opt()], outs=[xg[s].opt()])

        wqk_sb = consts.tile([128, KT, 512], F16)
        nc.sync.dma_start(out=wqk_sb,
                          in_=wqk_d.rearrange("(kt p) c -> p kt c", p=128))
        bqk_sb = consts.tile([128, 4], F32)
        nc.sync.dma_start(out=bqk_sb, in_=bqk_d.rearrange("(ch p) -> p ch",
                                                          p=128))
        wv_sb = consts.tile([128, KT, VW], F16)
        nc.sync.dma_start(out=wv_sb,
                          in_=wv_d.rearrange("(kt p) c -> p kt c", p=128))
        # heads 0+1 stacked on 128 partitions (paired out-proj), head 2 alone
        wout_pair = consts.tile([128, D], F16)
        nc.sync.dma_start(out=wout_pair, in_=wout_d[0:128, :])
        wout_h2 = consts.tile([64, D], F16)
        nc.sync.dma_start(out=wout_h2, in_=wout_d[128:192, :])

        # persistent activations: q^T/k^T chunks and v (+ones col)
        qk_sb = persist.tile([128, 4, t], F16)
        vaug_sb = persist.tile([128, n_tch, HPC, DK + 1], F16)
        nc.vector.memset(vaug_sb[:, :, :, DK:DK + 1], 1.0)

        with tc.tile_pool(name="xtp", bufs=3) as xtp, \
             tc.tile_pool(name="stps", bufs=2, space="PSUM") as stps, \
             tc.tile_pool(name="cpsp", bufs=2, space="PSUM") as cpsp, \
             tc.tile_pool(name="fillp", bufs=2, space="PSUM") as fillp, \
             tc.tile_pool(name="ptp", bufs=4) as ptp, \
             tc.tile_pool(name="ctxp", bufs=6) as ctxp, \
             tc.tile_pool(name="smp", bufs=4) as smp, \
             tc.tile_pool(name="outp", bufs=2) as outp:

            def load_xt(j):
                xt = xtp.tile([128, KT, ICH_W], F16, tag="xt")
                i0 = j * ICH_W
                if use_coll:
                    s, r0 = j // 2, 2 * (j % 2)
                    hw_ = ICH_W // 2
                    for rr in range(2):
                        nc.sync.dma_start(
                            out=xt[:, :, rr * hw_:(rr + 1) * hw_],
                            in_=xg[s, r0 + rr].rearrange("kt p w -> p kt w"))
                else:
                    src = x_d[:, :, i0:i0 + ICH_W].rearrange("kt p w -> p kt w")
                    nc.sync.dma_start(out=xt, in_=src)
                return xt

            def proj_thunks(j, xt):
                """Filler thunks computing chunk j's qkv projections."""
                i0 = j * ICH_W
                ths = []
                for ch in range(4):
                    def th(ch=ch):
                        qps = fillp.tile([128, 512], F32, tag="fill",
                                         space="PSUM")
                        for kt in range(KT):
                            nc.tensor.matmul(
                                qps,
                                lhsT=wqk_sb[:, kt, ch * 128:(ch + 1) * 128],
                                rhs=xt[:, kt, :],
                                start=(kt == 0), stop=(kt == KT - 1))
                        nc.vector.tensor_scalar_add(
                            qk_sb[:, ch, i0:i0 + ICH_W], qps,
                            bqk_sb[:, ch:ch + 1])
                    ths.append(th)
                for tl in range(4):
                    def th(tl=tl):
                        vps = fillp.tile([128, 512], F32, tag="fill",
                                         space="PSUM")
                        for kt in range(KT):
                            nc.tensor.matmul(
                                vps[:, 0:VW],
                                lhsT=xt[:, kt, tl * 128:(tl + 1) * 128],
                                rhs=wv_sb[:, kt, :],
                                start=(kt == 0), stop=(kt == KT - 1))
                        nc.vector.tensor_copy(
                            vaug_sb[:, j * 4 + tl, :, 0:DK],
                            vps[:, 0:VW].rearrange("p (h d) -> p h d", h=HPC))
                    ths.append(th)
                return ths

            def outproj_thunks(j, ctxn):
                """Filler thunks for chunk j's output projection (+ its RS).

                The last chunk's accumulators rotate through the (by then
                idle) cps PSUM ring instead of the single fill slot, so its
                8 projection chains pipeline instead of serializing."""
                i0 = j * ICH_W
                last = (j == n_ich - 1)
                ths = []
                osbs = {}
                for tsub in range(4):
                    for mh, (m0, m1) in enumerate(((0, 384), (384, D))):
                        def th(tsub=tsub, mh=mh, m0=m0, m1=m1):
                            if mh == 0:
                                osbs[tsub] = outp.tile([128, D], F16,
                                                       tag="osb", name="osb")
                            osb = osbs[tsub]
                            if last:
                                ops = cpsp.tile([128, 512], F32, tag="cps",
                                                space="PSUM", name="ops")
                            else:
                                ops = fillp.tile([128, 512], F32, tag="fill",
                                                 space="PSUM", name="ops")
                            ts_ = slice(tsub * 128, (tsub + 1) * 128)
                            nc.tensor.matmul(
                                ops[:, 0:384], lhsT=ctxn[0][:, ts_],
                                rhs=wout_pair[:, m0:m1],
                                start=True, stop=False)
                            nc.tensor.matmul(
                                ops[:, 0:384], lhsT=ctxn[2][:, ts_],
                                rhs=wout_h2[:, m0:m1],
                                start=False, stop=True)
                            nc.vector.tensor_copy(osb[:, m0:m1], ops[:, 0:384])
                            if mh == 1:
                                dst = ob if use_coll else out_d
                                r1 = i0 + (tsub + 1) * 128
                                nc.sync.dma_start(
                                    out=dst[r1 - 128:r1, :], in_=osb)
                                if use_coll and r1 in row_close:
                                    r0 = row_close[r1][0]
                                    q0, q1 = r0 // 4, r1 // 4
                                    nc.gpsimd.collective_compute(
                                        "ReduceScatter", mybir.AluOpType.add,
                                        replica_groups=GROUPS,
                                        ins=[ob[r0:r1, :].opt()],
                                        outs=[rsq[q0:q1, :].opt()])
                                    nc.sync.dma_start(out_d[q0:q1, :],
                                                      rsq[q0:q1, :])
                        ths.append(th)
                return ths

            def _norm_into(cps, cn_slice):
                den = smp.tile([1, ICH_W], F32, tag="den", name="den")
                nc.vector.tensor_copy(den, cps[DK:DK + 1, :])
                rec = smp.tile([1, ICH_W], F32, tag="rec", name="rec")
                nc.vector.reciprocal_approx_fast(out=rec, in_=den)
                rb = smp.tile([64, ICH_W], F32, tag="rb", name="rb")
                nc.gpsimd.partition_broadcast(rb, rec)
                nc.vector.tensor_mul(cn_slice, cps[0:DK, :], rb)

            def normalize_pair(cps_a, cps_b):
                # h0 -> partitions 0:64, h1 -> 64:128 of one tile so the
                # out-projection contracts both heads in a single K=128 MM
                cn = ctxp.tile([128, ICH_W], F16, tag="ctxn2", name="cn")
                _norm_into(cps_a, cn[0:DK, :])
                _norm_into(cps_b, cn[DK:2 * DK, :])
                return cn

            def normalize(cps):
                cn = ctxp.tile([64, ICH_W], F16, tag="ctxn", name="cn")
                _norm_into(cps, cn)
                return cn

            pending = []

            def drain(k):
                for _ in range(k):
                    if pending:
                        pending.pop(0)()

            xt = load_xt(0)
            for th in proj_thunks(0, xt):
                th()

            for ich in range(n_ich):
                i0 = ich * ICH_W
                njb = (i0 + ICH_W) // JB_W
                if ich + 1 < n_ich:
                    xt = load_xt(ich + 1)
                    pending.extend(proj_thunks(ich + 1, xt))

                # ---- pass A: heads 0 and 1, row-group paired ----
                cps0 = cpsp.tile([DK + 1, ICH_W], F32, tag="cps", space="PSUM")
                cps1 = cpsp.tile([DK + 1, ICH_W], F32, tag="cps", space="PSUM")

                def q_lo_of(jb):
                    # queries below the diagonal band are fully masked —
                    # skip them in QK, exp, select and PV
                    s = jb - (njb - 4)
                    return 128 * s if s > 0 else 0

                def pv_a(pt, jb):
                    ql = q_lo_of(jb)
                    nc.tensor.matmul(
                        cps0[:, ql:], lhsT=vaug_sb[:, jb, 0, :],
                        rhs=pt[:, 0, ql:],
                        start=(jb == 0), stop=(jb == njb - 1))
                    nc.tensor.matmul(
                        cps1[:, ql:], lhsT=vaug_sb[:, jb, 1, :],
                        rhs=pt[:, 1, ql:],
                        start=(jb == 0), stop=(jb == njb - 1))

                prev = None     # PV runs one j-block behind QK/exp so the
                for jb in range(njb):   # in-order PE queue never waits on ACT
                    j0 = jb * JB_W
                    ql = q_lo_of(jb)
                    st = stps.tile([128, 2, ICH_W], F32, tag="st",
                                   space="PSUM")
                    nc.tensor.matmul(
                        st[:, 0, ql:], lhsT=qk_sb[0:64, 1, j0:j0 + JB_W],
                        rhs=qk_sb[0:64, 0, i0 + ql:i0 + ICH_W],
                        start=True, stop=True)
                    nc.tensor.matmul(
                        st[:, 1, ql:], lhsT=qk_sb[64:128, 1, j0:j0 + JB_W],
                        rhs=qk_sb[64:128, 0, i0 + ql:i0 + ICH_W],
                        start=True, stop=True)
                    pt = ptp.tile([128, 2, ICH_W], F16, tag="pt")
                    nc.scalar.activation(pt[:, :, ql:], st[:, :, ql:],
                                         mybir.ActivationFunctionType.Exp,
                                         bias=0.0, scale=1.0 / np.sqrt(DK))
                    s = jb - (njb - 4)          # diag position if >= 0
                    if s >= 0:
                        for hh in range(2):
                            nc.gpsimd.affine_select(
                                out=pt[:, hh, ql:ql + JB_W],
                                in_=pt[:, hh, ql:ql + JB_W],
                                compare_op=mybir.AluOpType.is_ge,
                                fill=0.0, base=0,
                                pattern=[[1, JB_W]], channel_multiplier=-1)
                    if njb <= 12 or jb % 2 == 1:
                        drain(1)
                    if prev is not None:
                        pv_a(*prev)
                    prev = (pt, jb)
                pv_a(*prev)

                ctxn = {0: normalize_pair(cps0, cps1)}

                # ---- pass B: head 2, alternating row groups ----
                cps2 = cpsp.tile([DK + 1, ICH_W], F32, tag="cps", space="PSUM")

                def pv_b(pt, grp):
                    for jj in range(2):
                        jb = grp * 2 + jj
                        ql = q_lo_of(jb)
                        nc.tensor.matmul(
                            cps2[:, ql:], lhsT=vaug_sb[:, jb, 2, :],
                            rhs=pt[:, jj, ql:],
                            start=(jb == 0), stop=(jb == njb - 1))

                prev = None
                for grp in range(njb // 2):
                    st = stps.tile([128, 2, ICH_W], F32, tag="st",
                                   space="PSUM")
                    for jj in range(2):
                        jb = grp * 2 + jj
                        j0 = jb * JB_W
                        ql = q_lo_of(jb)
                        if jb % 2 == 0:
                            lhsT = qk_sb[0:64, 3, j0:j0 + JB_W]
                            rhs = qk_sb[0:64, 2, i0 + ql:i0 + ICH_W]
                        else:
                            lhsT = qk_sb[64:128, 2, j0:j0 + JB_W]
                            rhs = qk_sb[64:128, 3, i0 + ql:i0 + ICH_W]
                        nc.tensor.matmul(st[:, jj, ql:], lhsT=lhsT, rhs=rhs,
                                         start=True, stop=True)
                    pt = ptp.tile([128, 2, ICH_W], F16, tag="pt")
                    ql0, ql1 = q_lo_of(grp * 2), q_lo_of(grp * 2 + 1)
                    if ql0 == ql1:
                        nc.scalar.activation(pt[:, :, ql0:], st[:, :, ql0:],
                                             mybir.ActivationFunctionType.Exp,
                                             bias=0.0, scale=1.0 / np.sqrt(DK))
                    else:
                        for jj, qlj in ((0, ql0), (1, ql1)):
                            nc.scalar.activation(
                                pt[:, jj, qlj:], st[:, jj, qlj:],
                                mybir.ActivationFunctionType.Exp,
                                bias=0.0, scale=1.0 / np.sqrt(DK))
                    for jj in range(2):
                        jb = grp * 2 + jj
                        s = jb - (njb - 4)
                        if s >= 0:
                            ql = q_lo_of(jb)
                            nc.gpsimd.affine_select(
                                out=pt[:, jj, ql:ql + JB_W],
                                in_=pt[:, jj, ql:ql + JB_W],
                                compare_op=mybir.AluOpType.is_ge,
                                fill=0.0, base=0,
                                pattern=[[1, JB_W]], channel_multiplier=-1)
                    drain(1)
                    if prev is not None:
                        pv_b(*prev)
                    prev = (pt, grp)
                pv_b(*prev)

                ctxn[2] = normalize(cps2)

                # drain leftovers (proj j+1 must be fully emitted before the
                # next chunk's attention reads qk_sb at its own columns)
                drain(len(pending))
                pending.extend(outproj_thunks(ich, ctxn))

            drain(len(pending))

    nc.compile()
    return nc


def make_core_inputs(x_full, W_qkv, b_qkv, W_out, b, hg, t=T,
                     use_coll=USE_COLL):
    """Host-side input prep for core (b, hg): fp16 shard + permuted weights."""
    if use_coll:
        sw = t // 16
        x_in = np.stack([
            np.ascontiguousarray(
                np.asarray(x_full[b][(4 * s + hg) * sw:(4 * s + hg + 1) * sw],
                           np.float32).T, np.float16).reshape(KT, 128, sw)
            for s in range(4)])
    else:
        xs = np.asarray(x_full[b], np.float32)
        x_in = np.ascontiguousarray(xs.T, np.float16).reshape(KT, 128, -1)

    heads = [hg * HPC + i for i in range(HPC)]
    # W_qkv last-dim layout: c = h*192 + s*64 + d  (s: 0=q 1=k 2=v)
    def cols(h, s):
        return slice(h * 192 + s * 64, h * 192 + s * 64 + 64)

    q = [np.asarray(W_qkv[:, cols(h, 0)]) for h in heads]
    k = [np.asarray(W_qkv[:, cols(h, 1)]) for h in heads]
    v = [np.asarray(W_qkv[:, cols(h, 2)]) for h in heads]
    bq = [np.asarray(b_qkv[cols(h, 0)]) for h in heads]

    wqk = np.concatenate([q[0], q[1], k[0], k[1], q[2], k[2], k[2], q[2]],
                         axis=1).astype(np.float16)
    z = np.zeros(64, np.float32)
    bqk = np.concatenate([bq[0], bq[1], z, z, bq[2], z, z, bq[2]]).astype(
        np.float32)
    wv = np.concatenate(v, axis=1).astype(np.float16)
    wout = np.concatenate(
        [np.asarray(W_out[h * DK:(h + 1) * DK, :]) for h in heads],
        axis=0).astype(np.float16)
    return {
        "x": np.ascontiguousarray(x_in),
        "wqk": np.ascontiguousarray(wqk),
        "bqk": np.ascontiguousarray(bqk),
        "wv": np.ascontiguousarray(wv),
        "wout": np.ascontiguousarray(wout),
    }


_CACHE = {}


def _get_program(t=T, use_coll=USE_COLL):
    key = (t, use_coll)
    if key not in _CACHE:
        _CACHE[key] = build_program(t, use_coll)
    return _CACHE[key]


def run_cores(inputs, t=T, trace=False):
    nc = _get_program(t)
    x = np.asarray(inputs["x"], np.float32)
    in_maps = []
    for core in range(N_CORES):
        b, hg = core // 4, core % 4
        in_maps.append(make_core_inputs(x, inputs["W_qkv"], inputs["b_qkv"],
                                        inputs["W_out"], b, hg, t=t))
    res = run_bass_kernel_spmd(nc, in_maps, list(range(N_CORES)), trace=trace)
    return res


def gather(inputs, results, t=T, use_coll=USE_COLL):
    b_qkv = np.asarray(inputs["b_qkv"], np.float32)
    W_out = np.asarray(inputs["W_out"], np.float32)
    b_out = np.asarray(inputs["b_out"], np.float32)
    bv = np.concatenate([b_qkv[h * 192 + 128:h * 192 + 192] for h in range(H)])
    fold = bv @ W_out + b_out                      # [D]
    n_ich = t // ICH_W
    out = np.zeros((B, t, D), np.float32)
    for core in range(N_CORES):
        b, c = core // 4, core % 4
        r = np.asarray(results[core]["out"], np.float16).astype(np.float32)
        if use_coll:
            # per RS row-window (r0,r1): core c holds rows
            # [r0 + c*q, +q) at output offset r0//4
            for r0, r1 in RS_ROWS:
                q = (r1 - r0) // 4
                t0 = r0 + c * q
                out[b, t0:t0 + q, :] = r[r0 // 4:r0 // 4 + q]
        else:
            out[b] += r
    out += fold[None, None, :]
    return out


def kernel(**inputs):
    res = run_cores(inputs)
    return gather(inputs, res.results)


if __name__ == "__main__":
    # smoke test with random data
    rng = np.random.default_rng(0)
    inputs = {
        "x": rng.standard_normal((B, T, D), dtype=np.float32),
        "mask": np.triu(np.ones((T, T), dtype=bool), k=1),
        "W_qkv": (rng.standard_normal((D, 3 * D), dtype=np.float32)
                  / np.sqrt(D)),
        "b_qkv": rng.standard_normal(3 * D).astype(np.float32) * 0.02,
        "W_out": (rng.standard_normal((D, D), dtype=np.float32)
                  / np.sqrt(D)),
        "b_out": rng.standard_normal(D).astype(np.float32) * 0.02,
    }
    out = kernel(**inputs)
    print(out.shape, out.dtype)
